# revision 11
# baseline (speedup 1.0000x reference)
# Trainium2 Bass kernel for NormalAttention (1x1-conv q/k/v attention over HW).
#
# Math (per batch b, one batch per NeuronCore):
#   q = Wq x + bq            [64, 4096]
#   k = Wk x + bk            [64, 4096]
#   v = Wv x + bv            [256, 4096]
#   E[n,m] = sum_c q[c,n] k[c,m]          (energy, [4096, 4096])
#   out = v @ (elu(E))/N                  [256, 4096]
#   y = Wg out + bg
#
# Key restructuring vs a straight flash-style evaluation:
#   elu(E)+1 = exp(min(E,0)) + relu(E) = [exp(-r) + r] + E,  r = relu(-E)
# so with w = exp(-r) + r (a function of the single tensor r):
#   v@(elu(E)+1) = v@w + v@E = v@w + (v q^T) k
# The v@E term collapses through associativity into a rank-64 path
# ("PK"): P2 = q v^T/N (tiny [64,256]), then P2^T @ k accumulated into
# the same PSUM banks as v@w.  The elementwise chain per E-tile is then
#   r = relu(-E)    (ACT or DVE, the only PSUM read)
#   t = exp(-r)     (ACT, SBUF)
#   w = t + r       (DVE or Pool tensor-tensor add, SBUF bf16)
# i.e. one PSUM read per element instead of two, no clamp op, and the
# combine is a plain add that the (slow but idle) Pool engine can take a
# share of.  The "+1" offset and all conv biases are folded exactly into
# rank-1 corrections through the gamma bias, as in:
#   y = Wg (v@w + P2^T k)/1 + (bg - Wg S),  S = rowsum(v)/N
import os

import numpy as np
import ml_dtypes

import concourse.bass as bass
import concourse.mybir as mybir
import concourse.tile as tile
from concourse import bacc
from concourse.bass_utils import run_bass_kernel_spmd

B, C, HH, WW = 8, 256, 64, 64
N = HH * WW          # 4096 spatial positions
CQ = 64              # query/key channels
NCORES = 8
MT = 512             # m (energy column) tile
NPAIRS = 16          # pairs of 128-row n-chunks per m-tile
NMT = N // MT        # 8 m-tiles

# engine-balance knobs: r-op j goes to ACT if j % R_MOD < R_THR (else DVE);
# w-op u goes to Pool if u % W_MOD < W_THR (else DVE).
R_MOD = int(os.environ.get("R_MOD", "20"))
R_THR = int(os.environ.get("R_THR", "7"))
W_MOD = int(os.environ.get("W_MOD", "2"))
W_THR = int(os.environ.get("W_THR", "1"))
QK16 = os.environ.get("QK16", "0") == "1"  # bf16 q/k (explicit LDW + FWL)

F32 = mybir.dt.float32
F32R = mybir.dt.float32r
BF16 = mybir.dt.bfloat16
AL = mybir.AluOpType
AF = mybir.ActivationFunctionType


def build_nc(reps=1, variant=None):
    nc = bacc.Bacc("TRN2", target_bir_lowering=False, debug=False,
                   num_devices=NCORES)
    xd = nc.declare_dram_parameter("x", [2, 128, N], F32R, isOutput=False)
    wqd = nc.declare_dram_parameter("wqT", [2, 128, CQ], F32R, isOutput=False)
    wkd = nc.declare_dram_parameter("wkT", [2, 128, CQ], F32R, isOutput=False)
    bqd = nc.declare_dram_parameter("bq", [CQ, 1], F32, isOutput=False)
    bkd = nc.declare_dram_parameter("bk", [CQ, 1], F32, isOutput=False)
    bqTd = nc.declare_dram_parameter("bqT16", [1, CQ], BF16, isOutput=False)
    wvd = nc.declare_dram_parameter("wvT", [2, 128, C], F32R, isOutput=False)
    bvd = nc.declare_dram_parameter("bv", [1, C], F32R, isOutput=False)
    wgd = nc.declare_dram_parameter("wgT16", [2, 128, C], BF16, isOutput=False)
    bgd = nc.declare_dram_parameter("bg", [C, 1], F32, isOutput=False)
    onesd = nc.declare_dram_parameter("ones", [1, 128], F32R, isOutput=False)
    od = nc.declare_dram_parameter("out", [2, 128, N], F32, isOutput=True)

    with tile.TileContext(nc) as tc:
        with (
            tc.tile_pool(name="wts", bufs=1) as wts,
            tc.tile_pool(name="xs", bufs=2) as xs,
            tc.tile_pool(name="qk", bufs=1) as qkp,
            tc.tile_pool(name="vt", bufs=1) as vtp,
            tc.tile_pool(name="elem", bufs=3) as elem,
            tc.tile_pool(name="finp", bufs=2) as finp,
            tc.tile_pool(name="pse", bufs=2, space="PSUM") as pse,
            tc.tile_pool(name="pso", bufs=1, space="PSUM") as pso,
            tc.tile_pool(name="psg", bufs=2, space="PSUM") as psg,
        ):
            def body(iv=None):
                x_sb = [xs.tile([128, N], F32R, tag=f"x{i}", name=f"x_sb{i}")
                        for i in range(2)]
                for i in range(2):
                    for cch in range(4):
                        cs = slice(cch * (N // 4), (cch + 1) * (N // 4))
                        nc.sync.dma_start(x_sb[i][:, cs], xd[i][:, cs])
                wq_sb = wts.tile([128, 2, CQ], F32R, tag="wq", name="wq_sb")
                wk_sb = wts.tile([128, 2, CQ], F32R, tag="wk", name="wk_sb")
                wv_sb = wts.tile([128, 2, C], F32R, tag="wv", name="wv_sb")
                wg_sb = wts.tile([128, 2, C], BF16, tag="wg", name="wg_sb")
                for i in range(2):
                    nc.sync.dma_start(wq_sb[:, i, :], wqd[i])
                    nc.sync.dma_start(wk_sb[:, i, :], wkd[i])
                    nc.sync.dma_start(wv_sb[:, i, :], wvd[i])
                    nc.sync.dma_start(wg_sb[:, i, :], wgd[i])
                bq_sb = wts.tile([CQ, 1], F32, tag="bq", name="bq_sb")
                nc.sync.dma_start(bq_sb, bqd[:])
                bk_sb = wts.tile([CQ, 1], F32, tag="bk", name="bk_sb")
                nc.sync.dma_start(bk_sb, bkd[:])
                bqT_sb = wts.tile([1, CQ], BF16, tag="bqT", name="bqT_sb")
                nc.sync.dma_start(bqT_sb, bqTd[:])
                bv_sb = wts.tile([1, C], F32R, tag="bv", name="bv_sb")
                nc.sync.dma_start(bv_sb, bvd[:])
                bg_sb = wts.tile([128, 2], F32, tag="bg", name="bg_sb")
                for h in range(2):
                    nc.sync.dma_start(bg_sb[:, h:h + 1],
                                      bgd[h * 128:(h + 1) * 128, :])
                ones_row = wts.tile([1, 128], F32R, tag="ones_row",
                                    name="ones_row")
                nc.sync.dma_start(ones_row, onesd[:])
                ones_col = wts.tile([128, 1], BF16, tag="ones_col",
                                    name="ones_col")
                nc.vector.memset(ones_col, 1.0)

                QKDT = BF16 if QK16 else F32R
                q_sb = qkp.tile([2 * CQ, N], QKDT, tag="q", name="q_sb")
                k_sb = qkp.tile([2 * CQ, N], QKDT, tag="k", name="k_sb")
                qT_sb = vtp.tile([128, 32, CQ], BF16, tag="qT", name="qT_sb")
                vt_sb = vtp.tile([128, 32, C], BF16, tag="vt", name="vt_sb")
                P2_sb = wts.tile([128, C], BF16 if QK16 else F32R,
                                 tag="P2", name="P2_sb")
                sT_sb = wts.tile([1, C], F32, tag="sT", name="sT_sb")
                sT16 = wts.tile([1, C], BF16, tag="sT16", name="sT16")
                s_col = wts.tile([128, 2], BF16, tag="scol", name="s_col")
                bge_sb = wts.tile([128, 2], F32, tag="bge", name="bge_sb")

                # ---- q, k = conv1x1(x) + bias   [64, 4096] row-form ----
                for ti in range(NMT):
                    sl = slice(ti * 512, (ti + 1) * 512)
                    for qi, (dst, w_s, b_s) in enumerate(
                            ((q_sb, wq_sb, bq_sb), (k_sb, wk_sb, bk_sb))):
                        ps = psg.tile([CQ, 512], F32, tag="gps", name="qkps")
                        nc.tensor.matmul(ps, w_s[:, 0, :], x_sb[0][:, sl],
                                         start=True, stop=False)
                        nc.tensor.matmul(ps, w_s[:, 1, :], x_sb[1][:, sl],
                                         start=False, stop=True)
                        if (ti + qi) % 2 == 0:
                            nc.scalar.activation(dst[:CQ, sl], ps,
                                                 AF.Identity, bias=b_s,
                                                 scale=1.0)
                        else:
                            nc.vector.tensor_scalar(dst[:CQ, sl], ps, b_s,
                                                    None, AL.add)
                # duplicate q/k into partitions 64..127 (PE row-group packing)
                for dst in (q_sb, k_sb):
                    nc.sync.dma_start(dst[CQ:2 * CQ, :], dst[:CQ, :])

                # ---- qT (unbiased) = x^T WqT, bf16 [4096, 64] ----
                # batched: 16 n-chunks per [128,1024] PSUM tile, 2 tiles.
                for half in range(2):
                    ps = pse.tile([128, 2 * MT], F32, tag="eps", name="qTps")
                    for sub in range(16):
                        ni = half * 16 + sub
                        nsl = slice(ni * 128, (ni + 1) * 128)
                        osl = slice(sub * CQ, (sub + 1) * CQ)
                        nc.tensor.matmul(ps[:, osl], x_sb[0][:, nsl],
                                         wq_sb[:, 0, :], start=True,
                                         stop=False)
                        nc.tensor.matmul(ps[:, osl], x_sb[1][:, nsl],
                                         wq_sb[:, 1, :], start=False,
                                         stop=True)
                    dst = qT_sb[:, half * 16:(half + 1) * 16, :].rearrange(
                        "p a b -> p (a b)")
                    if half == 0:
                        nc.scalar.activation(dst, ps, AF.Copy)
                    else:
                        nc.vector.tensor_copy(dst, ps)

                # ---- v^T = (x^T WvT + bv)/4096, bf16 [4096, 256] ----
                # batched: 4 n-chunks per [128,1024] PSUM tile, 8 tiles.
                for g4 in range(8):
                    ps = pse.tile([128, 2 * MT], F32, tag="eps", name="vps")
                    for sub in range(4):
                        ni = g4 * 4 + sub
                        nsl = slice(ni * 128, (ni + 1) * 128)
                        osl = slice(sub * C, (sub + 1) * C)
                        nc.tensor.matmul(ps[:, osl], x_sb[0][:, nsl],
                                         wv_sb[:, 0, :], start=True,
                                         stop=False)
                        nc.tensor.matmul(ps[:, osl], x_sb[1][:, nsl],
                                         wv_sb[:, 1, :], start=False,
                                         stop=False)
                        nc.tensor.matmul(ps[:, osl], ones_row, bv_sb,
                                         start=False, stop=True)
                    dst = vt_sb[:, g4 * 4:(g4 + 1) * 4, :].rearrange(
                        "p a b -> p (a b)")
                    if g4 % 2 == 0:
                        nc.scalar.activation(dst, ps, AF.Copy)
                    else:
                        nc.vector.tensor_copy(dst, ps)

                # ---- S = rowsum(v/4096); bg_eff = bg - Wg S ----
                sps = pso.tile([1, C], F32, tag="o0", name="sps")
                for ni in range(32):
                    nc.tensor.matmul(sps, ones_col, vt_sb[:, ni, :],
                                     start=(ni == 0), stop=(ni == 31))
                nc.vector.tensor_copy(sT_sb, sps)
                nc.vector.tensor_copy(sT16, sps)
                for h in range(2):
                    # [1,128] row -> [128,1] column (with f32->bf16 cast)
                    nc.gpsimd.dma_start(s_col[:, h:h + 1],
                                        sT_sb[:, h * 128:(h + 1) * 128])
                for h in range(2):
                    hsl = slice(h * 128, (h + 1) * 128)
                    ps = pso.tile([128, 1], F32, tag="o1", name="bgps")
                    nc.tensor.matmul(ps, wg_sb[:, 0, hsl], s_col[:, 0:1],
                                     start=True, stop=False)
                    nc.tensor.matmul(ps, wg_sb[:, 1, hsl], s_col[:, 1:2],
                                     start=False, stop=True)
                    nc.scalar.activation(bge_sb[:, h:h + 1], ps, AF.Identity,
                                         bias=bg_sb[:, h:h + 1], scale=-1.0)

                # ---- P2 = (q_biased v^T)/N  [64, 256] ----
                p2ps = pso.tile([CQ, C], F32, tag="o1", name="p2ps")
                for ni in range(32):
                    nc.tensor.matmul(p2ps, qT_sb[:, ni, :], vt_sb[:, ni, :],
                                     start=(ni == 0), stop=False)
                nc.tensor.matmul(p2ps, bqT_sb, sT16, start=False, stop=True)
                nc.vector.tensor_copy(P2_sb[:CQ, :], p2ps)
                nc.sync.dma_start(P2_sb[CQ:128, :], P2_sb[:CQ, :])

                # ---- main attention loop (software-pipelined) ----
                # pair j = mt*NPAIRS + p covers n-chunks (2p, 2p+1) x m-tile
                # mt.  Stages at iteration j:
                #   PE : E(j+2), out(j-3 pairs), PK/gamma at group edges
                #   ACT: r(j+1) [share], t(u=(j-1)/2 on odd j]
                #   DVE: r(j+1) [share], w(u=j/2-1 on even j]
                #   Pool: w share
                pairs = [(mt, p) for mt in range(NMT) for p in range(NPAIRS)]
                NP = len(pairs)
                eps_q = {}
                r_q = {}
                t_q = {}
                w_q = {}
                o_ps = {}

                def emit_e(j):
                    mt, p = pairs[j]
                    msl = slice(mt * MT, (mt + 1) * MT)
                    nA, nB = 2 * p, 2 * p + 1
                    eps = pse.tile([128, 2 * MT], F32, tag="eps", name="eps")
                    nc.tensor.matmul(eps[:, 0:MT],
                                     q_sb[:CQ, nA * 128:(nA + 1) * 128],
                                     k_sb[:CQ, msl], start=True, stop=True)
                    nc.tensor.matmul(eps[:, MT:2 * MT],
                                     q_sb[CQ:2 * CQ, nB * 128:(nB + 1) * 128],
                                     k_sb[CQ:2 * CQ, msl],
                                     start=True, stop=True)
                    eps_q[j] = eps

                def emit_r(j):
                    u, half = j // 2, j % 2
                    if half == 0:
                        r_q[u] = elem.tile([128, 4 * MT], BF16, tag="r",
                                           name="r16")
                    rt = r_q[u]
                    hs = slice(half * 2 * MT, (half + 1) * 2 * MT)
                    eps = eps_q.pop(j)
                    if j % R_MOD < R_THR:
                        nc.scalar.activation(rt[:, hs], eps, AF.Relu,
                                             scale=-1.0)
                    else:
                        nc.vector.tensor_scalar(rt[:, hs], eps, -1.0,
                                                0.0, AL.mult, AL.max)

                def emit_t(u):
                    t16 = elem.tile([128, 4 * MT], BF16, tag="t",
                                    name="t16")
                    nc.scalar.activation(t16, r_q[u], AF.Exp, scale=-1.0)
                    t_q[u] = t16

                def emit_w(u):
                    w16 = elem.tile([128, 4 * MT], BF16, tag="w",
                                    name="w16")
                    rt = r_q.pop(u)
                    t16 = t_q.pop(u)
                    if u % W_MOD < W_THR:
                        nc.gpsimd.tensor_tensor(w16, t16, rt, AL.add)
                    else:
                        nc.vector.tensor_tensor(w16, t16, rt, AL.add)
                    w_q[u] = w16

                def emit_out(j):
                    mt, p = pairs[j]
                    nA, nB = 2 * p, 2 * p + 1
                    u, half = j // 2, j % 2
                    w16 = w_q[u]
                    if p == 0:
                        # open the PSUM accumulation group with the PK
                        # (low-rank v@E) matmuls
                        msl = slice(mt * MT, (mt + 1) * MT)
                        o_ps[mt] = []
                        for ci in range(2):
                            csl = slice(ci * 128, (ci + 1) * 128)
                            ps = pso.tile([128, MT], F32, tag=f"o{ci}",
                                          name=f"o_ps{ci}")
                            nc.tensor.matmul(
                                ps, P2_sb[ci * CQ:(ci + 1) * CQ, csl],
                                k_sb[ci * CQ:(ci + 1) * CQ, msl],
                                start=True, stop=False)
                            o_ps[mt].append(ps)
                    ho = half * 2 * MT
                    for ci in range(2):
                        csl = slice(ci * 128, (ci + 1) * 128)
                        nc.tensor.matmul(o_ps[mt][ci], vt_sb[:, nA, csl],
                                         w16[:, ho:ho + MT],
                                         start=False, stop=False)
                        nc.tensor.matmul(o_ps[mt][ci], vt_sb[:, nB, csl],
                                         w16[:, ho + MT:ho + 2 * MT],
                                         start=False, stop=(p == NPAIRS - 1))
                    if half == 1:
                        del w_q[u]

                def emit_gamma(mt):
                    msl = slice(mt * MT, (mt + 1) * MT)
                    osb = []
                    for ci in range(2):
                        ob = finp.tile([128, MT], BF16, tag=f"ob{ci}",
                                       name=f"ob{ci}")
                        if ci == 0:
                            nc.scalar.activation(ob, o_ps[mt][ci], AF.Copy)
                        else:
                            nc.vector.tensor_copy(ob, o_ps[mt][ci])
                        osb.append(ob)
                    del o_ps[mt]
                    for h in range(2):
                        hsl = slice(h * 128, (h + 1) * 128)
                        gps = psg.tile([128, MT], F32, tag="gps", name="gps")
                        nc.tensor.matmul(gps, wg_sb[:, 0, hsl], osb[0],
                                         start=True, stop=False)
                        nc.tensor.matmul(gps, wg_sb[:, 1, hsl], osb[1],
                                         start=False, stop=True)
                        fo = finp.tile([128, MT], F32, tag="fo", name="fo")
                        nc.scalar.activation(fo, gps, AF.Identity,
                                             bias=bge_sb[:, h:h + 1],
                                             scale=1.0)
                        nc.sync.dma_start(od[h, :, msl], fo)

                for s in range(NP + 5):
                    if s == 0:
                        emit_e(0)
                        emit_e(1)
                        emit_r(0)
                    if s + 2 < NP:
                        emit_e(s + 2)
                    if s + 1 < NP:
                        emit_r(s + 1)
                    if s % 2 == 1:
                        u = (s - 1) // 2
                        if 2 * u < NP:
                            emit_t(u)
                    if s % 2 == 0 and s >= 2:
                        u = s // 2 - 1
                        if 2 * u < NP:
                            emit_w(u)
                    if 0 <= s - 3 < NP:
                        emit_out(s - 3)
                        if pairs[s - 3][1] == NPAIRS - 1:
                            emit_gamma(pairs[s - 3][0])

            if reps == 1:
                body()
            else:
                with tc.For_i(0, reps, 1):
                    body()
    nc.compile()
    return nc


_NC_CACHE = {}


def _get_nc(reps=1, variant=None):
    key = (reps, variant)
    if key not in _NC_CACHE:
        _NC_CACHE[key] = build_nc(reps, variant)
    return _NC_CACHE[key]


def _prep_in_maps(inputs):
    x = np.ascontiguousarray(np.asarray(inputs["x"], dtype=np.float32))
    wq = np.asarray(inputs["query_weight"], np.float32)[:, :, 0, 0]
    bq = np.asarray(inputs["query_bias"], np.float32)
    wk = np.asarray(inputs["key_weight"], np.float32)[:, :, 0, 0]
    bk = np.asarray(inputs["key_bias"], np.float32)
    wv = np.asarray(inputs["value_weight"], np.float32)[:, :, 0, 0]
    bv = np.asarray(inputs["value_bias"], np.float32)
    wg = np.asarray(inputs["gamma_weight"], np.float32)[:, :, 0, 0]
    bg = np.asarray(inputs["gamma_bias"], np.float32)

    wqT = np.ascontiguousarray(wq.T).reshape(2, 128, CQ)
    wkT = np.ascontiguousarray(wk.T).reshape(2, 128, CQ)
    wvT = np.ascontiguousarray(wv.T / N).reshape(2, 128, C)
    bvr = (bv / N).reshape(1, C)
    wgT16 = np.ascontiguousarray(wg.T).astype(ml_dtypes.bfloat16).reshape(
        2, 128, C)
    shared = {
        "wqT": wqT, "wkT": wkT,
        "bq": np.ascontiguousarray(bq.reshape(CQ, 1)),
        "bk": np.ascontiguousarray(bk.reshape(CQ, 1)),
        "bqT16": bq.reshape(1, CQ).astype(ml_dtypes.bfloat16),
        "wvT": wvT, "bv": bvr, "wgT16": wgT16,
        "bg": np.ascontiguousarray(bg.reshape(C, 1)),
        "ones": np.ones((1, 128), np.float32),
    }
    return [dict(shared, x=x[b].reshape(2, 128, N))
            for b in range(B)]


def _run(inputs, trace=False, reps=1, variant=None):
    nc = _get_nc(reps, variant)
    in_maps = _prep_in_maps(inputs)
    res = run_bass_kernel_spmd(nc, in_maps, core_ids=list(range(NCORES)),
                               trace=trace)
    out = np.stack([r["out"].reshape(C, HH, WW) for r in res.results], axis=0)
    return out, res


def kernel(**inputs):
    out, _ = _run(inputs, trace=False)
    return out
